# revision 6
# baseline (speedup 1.0000x reference)
"""BiT-Phoneme dense transformer on 8 Trainium2 NeuronCores.

Sharding: data-parallel over batch (4) x sequence-split over token halves (2).
Core 2b+r handles batch b, query tokens [256r, 256r+256). Per layer, the two
cores of a pair AllGather their K/V halves (bf16) so every core attends over
the full 512-key range; causality and the relative-position bias are folded
into a per-core precomputed additive bias (masked entries = -1e9 -> exp = 0),
which also makes the SPMD program identical on every core.

Layout: activations live feature-major ([dim partitions, token free]) so every
matmul contracts over partitions with zero transposes. LayerNorm statistics are
computed with ones-vector matmuls over the partition axis; softmax runs with
keys on partitions (exp without max-subtraction -- scores are LN-bounded) and
per-query sums/reciprocals are broadcast back through K=1 matmuls.
"""

import os
import sys

sys.path.insert(0, "/opt/trn_rl_repo")

import numpy as np
import ml_dtypes

import concourse.bass as bass
import concourse.mybir as mybir
import concourse.tile as tile
from concourse import bacc
from concourse.bass_utils import run_bass_kernel_spmd

BF16 = ml_dtypes.bfloat16

P1, PW, DIM, HEADS, DH, MAXREL = 4, 256, 1024, 16, 64, 200
DEPTH = int(os.environ.get("BIT_DEPTH", "12"))
INNER = HEADS * DH
MLP = 4 * DIM
NCLS = 40
KSIZE, SIGMA = 20, 2.0
B, T = 4, 2048
N = T // P1            # 512 tokens (full sequence)
M = N // 2             # 256 tokens per core
NCORES = 8
GROUPS = [[0, 1], [2, 3], [4, 5], [6, 7]]
KCH = DIM // 128       # 8 feature chunks
MCH = MLP // 128       # 32 mlp chunks
KT = N // 128          # 4 key chunks
TCH = M // 128         # 2 token chunks per core
CONV_W = 96            # conv output tile width
NEG = -1e9

f32 = mybir.dt.float32
bf16 = mybir.dt.bfloat16
AL = mybir.AluOpType
AF = mybir.ActivationFunctionType


def _gauss_kernel():
    x = np.arange(KSIZE, dtype=np.float32)
    mean = (KSIZE - 1) / 2.0
    k = np.exp(-0.5 * ((x - mean) / SIGMA) ** 2) / (SIGMA * np.sqrt(2.0 * np.pi))
    return (k / k.sum()).astype(np.float32)


# ---------------------------------------------------------------- device code


def _emit_ln(nc, pools, chunks, gb, out_tile, sc):
    """LayerNorm over the feature axis (128 partitions x 8 chunks).

    chunks: 8 f32 APs [128, M]; gb(m) -> (gain [128,1], bias [128,1]);
    out_tile: [128, 8, M]; dtype conversion happens on the final write.
    """
    sbuf, small = pools["sbuf"], pools["small_ps"]

    stats = small.tile([1, 2 * M], f32, tag="small")
    for m in range(KCH):
        nc.tensor.matmul(stats[:, 0:M], sc["ones128_f"], chunks[m],
                         start=(m == 0), stop=(m == KCH - 1))
        sq = sbuf.tile([128, M], f32, tag="ln_sq")
        nc.vector.tensor_tensor(sq[:], chunks[m], chunks[m], AL.mult)
        nc.tensor.matmul(stats[:, M:2 * M], sc["ones128_f"], sq[:],
                         start=(m == 0), stop=(m == KCH - 1))

    vec = pools["vec1"].tile([1, 4 * M], f32, tag="ln_vec")
    mu, ex2, rinv, cmu = (vec[:, i * M:(i + 1) * M] for i in range(4))
    nc.vector.tensor_scalar_mul(mu, stats[:, 0:M], 1.0 / DIM)
    nc.vector.tensor_scalar_mul(ex2, stats[:, M:2 * M], 1.0 / DIM)
    var = pools["vec1"].tile([1, 2 * M], f32, tag="ln_var")
    nc.vector.tensor_tensor(var[:, 0:M], mu, mu, AL.mult)
    nc.vector.tensor_tensor(var[:, M:2 * M], ex2, var[:, 0:M], AL.subtract)
    nc.scalar.activation(var[:, 0:M], var[:, M:2 * M], AF.Sqrt, bias=sc["eps"][0:1, 1:2])
    nc.vector.reciprocal(rinv, var[:, 0:M])
    nc.vector.tensor_tensor(cmu, mu, rinv, AL.mult)
    nc.vector.tensor_scalar_mul(cmu, cmu, -1.0)

    bc = small.tile([128, 2 * M], f32, tag="small")
    nc.tensor.matmul(bc[:], sc["ones1_128"], vec[:, 2 * M:4 * M], start=True, stop=True)

    t1 = sbuf.tile([128, 2, M], f32, tag="ln_t1")
    for m in range(KCH):
        g, b = gb(m)
        nc.vector.tensor_tensor(t1[:, 0, :], chunks[m], bc[:, 0:M], AL.mult)
        nc.vector.tensor_tensor(t1[:, 1, :], t1[:, 0, :], bc[:, M:2 * M], AL.add)
        nc.vector.tensor_scalar(out_tile[:, m, :], t1[:, 1, :], g, b, AL.mult, AL.add)
    return out_tile


def _build_program():
    nc = bacc.Bacc(None, target_bir_lowering=False, num_devices=NCORES)

    xin = nc.dram_tensor("xin", [1152, PW], f32, kind="ExternalInput")
    biasT = nc.dram_tensor("biasT", [DEPTH, N, M], f32, kind="ExternalInput")
    gmat = nc.dram_tensor("gmat", [128, CONV_W], f32, kind="ExternalInput")
    patchw = nc.dram_tensor("patchw", [DIM, DIM], bf16, kind="ExternalInput")
    pvec = nc.dram_tensor("pvec", [5, DIM], f32, kind="ExternalInput")
    flnvec = nc.dram_tensor("flnvec", [2, DIM], f32, kind="ExternalInput")
    projw = nc.dram_tensor("projw", [DIM, 64], bf16, kind="ExternalInput")
    projb = nc.dram_tensor("projb", [128, 64], f32, kind="ExternalInput")
    lnvec = nc.dram_tensor("lnvec", [DEPTH, 6, DIM], f32, kind="ExternalInput")
    b1vec = nc.dram_tensor("b1vec", [DEPTH, MLP], f32, kind="ExternalInput")
    qkvwT = nc.dram_tensor("qkvwT", [DEPTH, DIM, 3 * INNER], bf16, kind="ExternalInput")
    outwT = nc.dram_tensor("outwT", [DEPTH, INNER, DIM], bf16, kind="ExternalInput")
    w1T = nc.dram_tensor("w1T", [DEPTH, DIM, MLP], bf16, kind="ExternalInput")
    w2T = nc.dram_tensor("w2T", [DEPTH, MLP, DIM], bf16, kind="ExternalInput")
    yout = nc.dram_tensor("yout", [M, NCLS + 1], f32, kind="ExternalOutput")

    KV_K = DIM * M
    KV_V = M * INNER
    half = KV_K + KV_V
    cin = nc.dram_tensor("cin", [half], bf16)
    cout = nc.dram_tensor("cout", [2 * half], bf16)

    with tile.TileContext(nc) as tc:
        with (
            tc.tile_pool(name="const", bufs=1) as const,
            tc.tile_pool(name="pool1", bufs=1) as pool1,
            tc.tile_pool(name="sbuf", bufs=3) as sbuf,
            tc.tile_pool(name="sbuf2", bufs=2) as sbuf2,
            tc.tile_pool(name="vec1", bufs=1) as vec1,
            tc.tile_pool(name="act", bufs=2) as act,
            tc.tile_pool(name="xres", bufs=2) as xres,
            tc.tile_pool(name="wpool", bufs=3) as wpool,
            tc.tile_pool(name="wvpool", bufs=1) as wvpool,
            tc.tile_pool(name="w2pool", bufs=2) as w2pool,
            tc.tile_pool(name="mm_ps", bufs=2, space="PSUM") as mm_ps,
            tc.tile_pool(name="st_ps", bufs=2, space="PSUM") as st_ps,
            tc.tile_pool(name="small_ps", bufs=2, space="PSUM") as small_ps,
        ):
            pools = dict(sbuf=sbuf, small_ps=small_ps, vec1=vec1)

            ones128_f = const.tile([128, 1], f32)
            nc.vector.memset(ones128_f[:], 1.0)
            ones128_b = const.tile([128, 1], bf16)
            nc.vector.memset(ones128_b[:], 1.0)
            ones1_128 = const.tile([1, 128], f32)
            nc.vector.memset(ones1_128[:], 1.0)
            ones1_64 = const.tile([1, 64], f32)
            nc.vector.memset(ones1_64[:], 1.0)
            czero = const.tile([128, 2], f32)
            nc.vector.memset(czero[:, 0:1], 0.0)
            nc.vector.memset(czero[:, 1:2], 1e-5)
            sc = dict(ones128_f=ones128_f[:], ones1_128=ones1_128[:], eps=None)

            sc["eps"] = czero[:]
            gm = const.tile([128, CONV_W], f32)
            nc.sync.dma_start(gm[:], gmat[:])
            pw = wvpool.tile([128, KCH, DIM], bf16, tag="wv", name="pw")
            nc.sync.dma_start(pw[:], patchw.ap().rearrange("(ko p) c -> p ko c", p=128))
            pv = const.tile([128, 5, KCH], f32)
            nc.sync.dma_start(pv[:], pvec.ap().rearrange("v (o p) -> p v o", p=128))
            fv = const.tile([128, 2, KCH], f32)
            nc.sync.dma_start(fv[:], flnvec.ap().rearrange("v (o p) -> p v o", p=128))
            pjw = const.tile([128, KCH, 64], bf16)
            nc.sync.dma_start(pjw[:], projw.ap().rearrange("(ko p) c -> p ko c", p=128))
            pjb = const.tile([128, 64], f32)
            nc.sync.dma_start(pjb[:], projb.ap())
            lnv = const.tile([128, DEPTH, 6, KCH], f32)
            nc.sync.dma_start(lnv[:], lnvec.ap().rearrange("l v (o p) -> p l v o", p=128))
            b1v = const.tile([128, DEPTH, MCH], f32)
            nc.sync.dma_start(b1v[:], b1vec.ap().rearrange("l (o p) -> p l o", p=128))

            # ---- Gaussian smoothing conv (fp32) -> convT [2][128c, 1024t]
            convT = [pool1.tile([128, P1 * M], f32, tag=f"convT{c}", name=f"convT{c}")
                     for c in range(2)]
            n_win = (P1 * M + CONV_W - 1) // CONV_W
            for w in range(n_win):
                tw = min(CONV_W, P1 * M - w * CONV_W)
                xw = sbuf.tile([128, PW], f32, tag="xw")
                nc.sync.dma_start(xw[:], xin[w * CONV_W:w * CONV_W + 128, :])
                for c in range(2):
                    ps = mm_ps.tile([128, 512], f32, tag="mm")
                    nc.tensor.matmul(ps[:, :tw], xw[:, c * 128:(c + 1) * 128], gm[:, :tw],
                                     start=True, stop=True)
                    nc.vector.tensor_copy(convT[c][:, w * CONV_W:w * CONV_W + tw], ps[:, :tw])

            # patch-major views: chunk kc of v.T -> convT[kc%2][:, (kc//2)::4]
            vviews = [convT[kc % 2].rearrange("p (t i) -> p i t", i=P1)[:, kc // 2, :]
                      for kc in range(KCH)]

            h1 = act.tile([128, KCH, M], bf16, tag="hT")
            _emit_ln(nc, pools, vviews,
                     lambda m: (pv[:, 0, m:m + 1], pv[:, 1, m:m + 1]), h1, sc)

            emb = pool1.tile([128, KCH, M], f32, tag="emb")
            for m in range(KCH):
                ps = mm_ps.tile([128, 512], f32, tag="mm")
                for k in range(KCH):
                    nc.tensor.matmul(ps[:, :M], pw[:, k, m * 128:(m + 1) * 128], h1[:, k, :],
                                     start=(k == 0), stop=(k == KCH - 1))
                nc.scalar.activation(emb[:, m, :], ps[:, :M], AF.Identity,
                                     bias=pv[:, 2, m:m + 1])
            xT = xres.tile([128, KCH, M], f32, tag="xT")
            _emit_ln(nc, pools, [emb[:, m, :] for m in range(KCH)],
                     lambda m: (pv[:, 3, m:m + 1], pv[:, 4, m:m + 1]), xT, sc)

            qkv_v = qkvwT.ap().rearrange("l (ko p) c -> l p ko c", p=128)
            outw_v = outwT.ap().rearrange("l (io p) c -> l p io c", p=128)
            w1_v = w1T.ap().rearrange("l (ko p) c -> l p ko c", p=128)
            w2_v = w2T.ap().rearrange("l (mo p) c -> l p mo c", p=128)

            for l in range(DEPTH):
                hT = act.tile([128, KCH, M], bf16, tag="hT")
                _emit_ln(nc, pools, [xT[:, m, :] for m in range(KCH)],
                         lambda m: (lnv[:, l, 0, m:m + 1], lnv[:, l, 1, m:m + 1]), hT, sc)

                # K projection -> kT (bf16, feature-major)
                kT = act.tile([128, KCH, M], bf16, tag="kT")
                for m in range(KCH):
                    wk = wpool.tile([128, KCH, 128], bf16, tag="wqk")
                    nc.sync.dma_start(wk[:], qkv_v[l][:, :, INNER + m * 128:INNER + (m + 1) * 128])
                    ps = mm_ps.tile([128, 512], f32, tag="mm")
                    for k in range(KCH):
                        nc.tensor.matmul(ps[:, :M], wk[:, k, :], hT[:, k, :],
                                         start=(k == 0), stop=(k == KCH - 1))
                    nc.vector.tensor_copy(kT[:, m, :], ps[:, :M])

                # V projection -> vmine (bf16, token-major)
                vmine = act.tile([128, TCH, INNER], bf16, tag="vmine")
                wv = wvpool.tile([128, KCH, INNER], bf16, tag="wv")
                nc.sync.dma_start(wv[:], qkv_v[l][:, :, 2 * INNER:3 * INNER])
                for th in range(TCH):
                    for nf in range(2):
                        ps = mm_ps.tile([128, 512], f32, tag="mm")
                        for k in range(KCH):
                            nc.tensor.matmul(ps[:],
                                             hT[:, k, th * 128:(th + 1) * 128],
                                             wv[:, k, nf * 512:(nf + 1) * 512],
                                             start=(k == 0), stop=(k == KCH - 1))
                        nc.vector.tensor_copy(vmine[:, th, nf * 512:(nf + 1) * 512], ps[:])

                # ship my K/V halves; AllGather within the pair
                nc.sync.dma_start(
                    cin[0:KV_K].rearrange("(m p q) -> p m q", m=KCH, p=128), kT[:])
                nc.sync.dma_start(
                    cin[KV_K:half].rearrange("(t p f) -> p t f", t=TCH, p=128), vmine[:])
                nc.gpsimd.collective_compute(
                    "AllGather", AL.bypass, replica_groups=GROUPS,
                    ins=[cin.ap()], outs=[cout.ap()])

                # Q projection (overlaps the collective); attention scale folded in
                qT = act.tile([128, KCH, M], bf16, tag="qT")
                for m in range(KCH):
                    wq = wpool.tile([128, KCH, 128], bf16, tag="wqk")
                    nc.sync.dma_start(wq[:], qkv_v[l][:, :, m * 128:(m + 1) * 128])
                    ps = mm_ps.tile([128, 512], f32, tag="mm")
                    for k in range(KCH):
                        nc.tensor.matmul(ps[:, :M], wq[:, k, :], hT[:, k, :],
                                         start=(k == 0), stop=(k == KCH - 1))
                    nc.vector.tensor_scalar_mul(qT[:, m, :], ps[:, :M], DH ** -0.5)

                # gathered K/V (rank-major == global token order)
                kfull = pool1.tile([128, KCH, N], bf16, tag="kfull")
                vfull = pool1.tile([128, KT, INNER], bf16, tag="vfull")
                for r in range(2):
                    nc.sync.dma_start(
                        kfull[:, :, r * M:(r + 1) * M],
                        cout[r * half:r * half + KV_K].rearrange(
                            "(m p q) -> p m q", m=KCH, p=128))
                    nc.sync.dma_start(
                        vfull[:, 2 * r:2 * r + 2, :],
                        cout[r * half + KV_K:(r + 1) * half].rearrange(
                            "(t p f) -> p t f", t=TCH, p=128))

                bias_sb = sbuf2.tile([128, KT, M], f32, tag="bias")
                nc.sync.dma_start(bias_sb[:], biasT[l].rearrange("(kt p) q -> p kt q", p=128))

                oT = act.tile([128, KCH, M], bf16, tag="oT")
                for h in range(HEADS):
                    hc, hp = h // 2, 64 * (h % 2)
                    st = st_ps.tile([128, KT, M], f32, tag="st")
                    for kt in range(KT):
                        nc.tensor.matmul(st[:, kt, :],
                                         kfull[hp:hp + 64, hc, kt * 128:(kt + 1) * 128],
                                         qT[hp:hp + 64, hc, :],
                                         start=True, stop=True)
                    scs = sbuf2.tile([128, KT, M], f32, tag="sc")
                    nc.vector.tensor_tensor(scs[:], st[:], bias_sb[:], AL.add)
                    pt = sbuf2.tile([128, KT, M], bf16, tag="pt")
                    nc.scalar.activation(pt[:], scs[:], AF.Exp, bias=czero[:, 0:1])

                    sums = small_ps.tile([1, M], f32, tag="small")
                    for kt in range(KT):
                        nc.tensor.matmul(sums[:], ones128_b[:], pt[:, kt, :],
                                         start=(kt == 0), stop=(kt == KT - 1))
                    rec = vec1.tile([1, M], f32, tag="rec")
                    nc.vector.reciprocal(rec[:], sums[:])
                    bc = small_ps.tile([64, M], f32, tag="small")
                    nc.tensor.matmul(bc[:], ones1_64[:], rec[:], start=True, stop=True)

                    av = small_ps.tile([64, M], f32, tag="small")
                    for kt in range(KT):
                        nc.tensor.matmul(av[:], vfull[:, kt, h * 64:(h + 1) * 64],
                                         pt[:, kt, :],
                                         start=(kt == 0), stop=(kt == KT - 1))
                    bcs = sbuf.tile([64, M], f32, tag="bcs")
                    nc.vector.tensor_copy(bcs[:], bc[:])
                    nc.vector.tensor_tensor(oT[hp:hp + 64, hc, :], av[:], bcs[:], AL.mult)

                # output projection + residual
                x2 = xres.tile([128, KCH, M], f32, tag="xT")
                for m in range(KCH):
                    wo = wpool.tile([128, KCH, 128], bf16, tag="wqk")
                    nc.sync.dma_start(wo[:], outw_v[l][:, :, m * 128:(m + 1) * 128])
                    ps = mm_ps.tile([128, 512], f32, tag="mm")
                    for i in range(KCH):
                        nc.tensor.matmul(ps[:, :M], wo[:, i, :], oT[:, i, :],
                                         start=(i == 0), stop=(i == KCH - 1))
                    tmp = sbuf.tile([128, M], f32, tag="rtmp")
                    nc.scalar.activation(tmp[:], ps[:, :M], AF.Identity,
                                         bias=lnv[:, l, 4, m:m + 1])
                    nc.vector.tensor_tensor(x2[:, m, :], tmp[:], xT[:, m, :], AL.add)

                # FFN
                hfT = act.tile([128, KCH, M], bf16, tag="hT")
                _emit_ln(nc, pools, [x2[:, m, :] for m in range(KCH)],
                         lambda m: (lnv[:, l, 2, m:m + 1], lnv[:, l, 3, m:m + 1]), hfT, sc)
                h1T = pool1.tile([128, MCH, M], bf16, tag="h1T")
                for mm in range(MCH):
                    w1 = wpool.tile([128, KCH, 128], bf16, tag="wqk")
                    nc.sync.dma_start(w1[:], w1_v[l][:, :, mm * 128:(mm + 1) * 128])
                    ps = mm_ps.tile([128, 512], f32, tag="mm")
                    for k in range(KCH):
                        nc.tensor.matmul(ps[:, :M], w1[:, k, :], hfT[:, k, :],
                                         start=(k == 0), stop=(k == KCH - 1))
                    nc.scalar.activation(h1T[:, mm, :], ps[:, :M], AF.Gelu,
                                         bias=b1v[:, l, mm:mm + 1])
                x3 = xres.tile([128, KCH, M], f32, tag="xT")
                for m in range(KCH):
                    ps = mm_ps.tile([128, 512], f32, tag="mm")
                    for hh in range(2):
                        w2 = w2pool.tile([128, MCH // 2, 128], bf16, tag="w2")
                        nc.sync.dma_start(
                            w2[:], w2_v[l][:, hh * 16:(hh + 1) * 16, m * 128:(m + 1) * 128])
                        for k in range(MCH // 2):
                            kk = hh * 16 + k
                            nc.tensor.matmul(ps[:, :M], w2[:, k, :], h1T[:, kk, :],
                                             start=(kk == 0), stop=(kk == MCH - 1))
                    tmp = sbuf.tile([128, M], f32, tag="rtmp")
                    nc.scalar.activation(tmp[:], ps[:, :M], AF.Identity,
                                         bias=lnv[:, l, 5, m:m + 1])
                    nc.vector.tensor_tensor(x3[:, m, :], tmp[:], x2[:, m, :], AL.add)
                xT = x3

            # ---- final LN + classifier head
            xf = act.tile([128, KCH, M], bf16, tag="hT")
            _emit_ln(nc, pools, [xT[:, m, :] for m in range(KCH)],
                     lambda m: (fv[:, 0, m:m + 1], fv[:, 1, m:m + 1]), xf, sc)
            for th in range(TCH):
                ps = mm_ps.tile([128, 512], f32, tag="mm")
                for k in range(KCH):
                    nc.tensor.matmul(ps[:, :64], xf[:, k, th * 128:(th + 1) * 128],
                                     pjw[:, k, :], start=(k == 0), stop=(k == KCH - 1))
                outt = sbuf.tile([128, 64], f32, tag="outt")
                nc.vector.tensor_tensor(outt[:], ps[:, :64], pjb[:], AL.add)
                nc.sync.dma_start(yout[th * 128:(th + 1) * 128, :], outt[:, :NCLS + 1])

    nc.compile()
    return nc


# ------------------------------------------------------------------ host side

_PROGRAM = None


def _get_program():
    global _PROGRAM
    if _PROGRAM is None:
        _PROGRAM = _build_program()
    return _PROGRAM


def _prep_shared(weights):
    w = {}
    gk = _gauss_kernel()
    G = np.zeros((128, CONV_W), np.float32)
    for t in range(128):
        lo = max(0, t - KSIZE + 1)
        hi = min(CONV_W - 1, t)
        for j in range(lo, hi + 1):
            G[t, j] = gk[t - j]
    w["gmat"] = G
    w["patchw"] = np.ascontiguousarray(np.asarray(weights["patch_w"]).T).astype(BF16)
    w["pvec"] = np.stack([weights["patch_ln1_g"], weights["patch_ln1_b"],
                          weights["patch_b"], weights["patch_ln2_g"],
                          weights["patch_ln2_b"]]).astype(np.float32)
    w["flnvec"] = np.stack([weights["final_ln_g"], weights["final_ln_b"]]).astype(np.float32)
    pj = np.zeros((DIM, 64), np.float32)
    pj[:, :NCLS + 1] = np.asarray(weights["proj_w"]).T
    w["projw"] = pj.astype(BF16)
    pb = np.zeros((128, 64), np.float32)
    pb[:, :NCLS + 1] = np.asarray(weights["proj_b"])[None, :]
    w["projb"] = pb
    w["lnvec"] = np.ascontiguousarray(np.stack(
        [weights["attn_ln_g"], weights["attn_ln_b"],
         weights["ffn_ln_g"], weights["ffn_ln_b"],
         weights["out_b"], weights["ffn_b2"]], axis=1)[:DEPTH]).astype(np.float32)
    w["b1vec"] = np.asarray(weights["ffn_b1"])[:DEPTH].astype(np.float32)
    w["qkvwT"] = np.ascontiguousarray(
        np.asarray(weights["qkv_w"])[:DEPTH].transpose(0, 2, 1)).astype(BF16)
    w["outwT"] = np.ascontiguousarray(
        np.asarray(weights["out_w"])[:DEPTH].transpose(0, 2, 1)).astype(BF16)
    w["w1T"] = np.ascontiguousarray(
        np.asarray(weights["ffn_w1"])[:DEPTH].transpose(0, 2, 1)).astype(BF16)
    w["w2T"] = np.ascontiguousarray(
        np.asarray(weights["ffn_w2"])[:DEPTH].transpose(0, 2, 1)).astype(BF16)
    return w


def _prep_bias(rel_tab):
    """Per-role masked additive bias, transposed: biasT[l, k, q] (f32)."""
    out = []
    k = np.arange(N)[:, None]
    for r in range(2):
        qg = (M * r + np.arange(M))[None, :]
        rel = np.clip(qg - k, -(MAXREL - 1), MAXREL - 1) + MAXREL - 1
        bias = rel_tab[:DEPTH][:, rel]                    # [DEPTH, N, M]
        bias = np.where((k <= qg)[None], bias, NEG)
        out.append(np.ascontiguousarray(bias.astype(np.float32)))
    return out


def kernel(**inputs):
    inputs = {k: np.asarray(v) for k, v in inputs.items()}
    nc = _get_program()
    shared = _prep_shared(inputs)
    bias_by_role = _prep_bias(np.asarray(inputs["rel_tab"], np.float32))
    x = inputs["neuralInput"].astype(np.float32)

    in_maps = []
    for c in range(NCORES):
        b, r = c // 2, c % 2
        xin = np.zeros((1152, PW), np.float32)
        lo = 1024 * r - 9
        s, e = max(lo, 0), min(lo + 1088, T)
        xin[s - lo:e - lo] = x[b, s:e]
        m = dict(shared)
        m["xin"] = xin
        m["biasT"] = bias_by_role[r]
        in_maps.append(m)

    res = run_bass_kernel_spmd(nc, in_maps, core_ids=list(range(NCORES)))
    out = np.zeros((B, N, NCLS + 1), np.float32)
    for c in range(NCORES):
        b, r = c // 2, c % 2
        out[b, M * r:M * (r + 1)] = res.results[c]["yout"]
    return out
